# revision 36
# baseline (speedup 1.0000x reference)
"""Binary CNN (4x binarized conv + BN + ReLU + maxpool + FC) on 8 TRN2 cores.

Pure data parallelism: each NeuronCore processes 512 images
([512,1,6,128] -> [512,10]); host shards/gathers along batch.

Everything on-chip is mapped onto dense [K=128, M=128, N=384] float32r
matmuls (full-rate 1 col/cycle at N>=256, self-loading weights) via a
phase-interleaved channels-on-partitions layout, so no cross-partition
data movement is ever needed:

  x      chunked DMA (weights on the ACT DMA queue in parallel) +
         PE-transpose to [W=128 partitions, rows]
  conv1  (1->32ch, k=9, s=2, pad 4): 16 structured matmuls; M-tile m =
         (phase p)*32 + ch covers output positions j = 8T + 2p + b0;
         maxpool partners land in one 2-bank PSUM tile -> DVE
         reduce_max over the bank axis; bias+relu ts in 2 halves.
  L1     partitions (v, ic32), w1 = 4u + v, free (u in 0..8, row)
  conv2  (32->64, k=3, pad 1): per u, h-even/h-odd banks, main matmul
         + cross-phase correction each, weight-major over u-pairs;
         ScalarE relu+bias epilogue emitted mid-pair for overlap.
  L2     partitions (s, oc64), w2 = 2h + s, free (h in 0..16, row)
  conv3  (64->128, k=3, pad 1): per pooled output b one 2-bank tile
         (even/odd partner), 2 matmuls each + corrections, weight-major
         over b-pairs; DVE reduce_max pools, ScalarE relu in 2 halves.
  L3     partitions oc128, free (b in 0..16, row)
  conv4  (128->128, kernel (6,1) valid): 6 accumulating matmuls
         contracting H, both w-halves per weight; single batched
         ScalarE epilogue.
  FC     (2048->10): 16 accumulating matmuls into one PSUM bank;
         output written [10, n_img] (contiguous DMA), host transposes.

PSUM: one pool of [128,2,512] tiles x4 bufs (all 8 banks) -- every
phase gets depth-4 rotation, which keeps PE ~80% busy. Hard-won HW
constraints honored here: a DVE op may read only ONE non-scalar input
from PSUM (hence single-AP strided reduce_max for pooling, not
tensor_tensor over two banks); tile_position partial-K sub-matmuls and
scalar_tensor_tensor are dramatically slower on real HW than the cost
model suggests (3-4x regressions) -- corrections use full 128x128
zero-padded weight blocks instead; bf16 weights force separate
LDWEIGHTS instructions and measured slower than f32r self-loading.

BN + conv-bias + ReLU: BN folded to per-channel a*z + beta with the
sign of a baked into the (+-1) binarized weights and |a| carried
forward into the next layer's weights, so each layer epilogue is
relu(z + beta') -- one op on DVE (tensor_scalar add/max) or ScalarE
(activation Relu with per-partition bias). Pooling commutes with the
monotone epilogue so it runs on raw PSUM pre-activations.
"""

import os

import numpy as np

import concourse.bass as bass
from concourse import bacc
import concourse.mybir as mybir
from concourse.masks import make_identity
from concourse.tile import TileContext

F32 = mybir.dt.float32
F32R = mybir.dt.float32r
BF16 = mybir.dt.bfloat16

N_CORES = 8
B_TOTAL = 4096
B_CORE = B_TOTAL // N_CORES  # 512
H = 6
W = 128
G = 64            # images per tile
RT = G * H        # 384 rows per tile

# wpack free-dim offsets (columns of the [128, WCOLS] constant block)
OFF_A1 = 0            # 16 x [128,128]
OFF_W2 = OFF_A1 + 16 * 128   # 4 x [128,128]: Wa, Wm, Wo, Wp
OFF_W3 = OFF_W2 + 4 * 128    # 4 x [128,128]: We1, We2, Wo1, Wo2
OFF_W4 = OFF_W3 + 4 * 128    # 6 x [128,128]
OFF_FC = OFF_W4 + 6 * 128    # 16 x [128,10]
WCOLS = OFF_FC + 16 * 10     # 4000 (bf16 weight block)
# cpack (fp32): identity 128 cols + 9 bias cols
OFF_IDENT = 0
OFF_BIAS = 128
CCOLS = 137


def _r(ap):
    """No-op for already-f32r APs (kept for call-site clarity)."""
    return ap


def build_nc(n_img=B_CORE, reps=1):
    assert n_img % G == 0
    nt = n_img // G
    rows = n_img * H
    nchunk = rows // 128

    nc = bacc.Bacc()
    x_h = nc.declare_dram_parameter("x", [rows, W], F32, isOutput=False)
    w_h = nc.declare_dram_parameter("wpack", [128, WCOLS], F32R, isOutput=False)
    i_h = nc.declare_dram_parameter("cpack", [128, CCOLS], F32, isOutput=False)
    out_h = nc.declare_dram_parameter("out", [10, n_img], F32, isOutput=True)

    add_op = mybir.AluOpType.add
    max_op = mybir.AluOpType.max

    with TileContext(nc) as tc:
        with (
            tc.tile_pool(name="singles", bufs=1) as singles,
            tc.tile_pool(name="a1", bufs=2) as a1_pool,
            tc.tile_pool(name="a2", bufs=2) as a2_pool,
            tc.tile_pool(name="a3", bufs=1) as a3_pool,
            tc.tile_pool(name="scr", bufs=2) as scr,
            tc.tile_pool(name="ps", bufs=4, space="PSUM") as ps,
        ):
            # DMA order matters for the first-exec critical path: ident +
            # first x chunk feed the transposes; weights are needed later.
            cp = singles.tile([128, CCOLS], F32)
            nc.sync.dma_start(out=cp, in_=i_h[:, :])
            ident = cp[:, OFF_IDENT:OFF_IDENT + 128]
            bt = cp[:, OFF_BIAS:OFF_BIAS + 9]
            act4 = singles.tile([128, 16, n_img], F32R)
            xall = singles.tile([128, nchunk, 128], F32)
            xv = x_h[:, :].rearrange("(c p) w -> p c w", p=128)
            xgrp = min(12, nchunk)
            nc.sync.dma_start(
                out=xall[:, :xgrp, :], in_=xv[:, :xgrp, :]
            )
            # weights go down the Activation engine's DMA queue so they
            # stream in parallel with x on the SP queue
            wt = singles.tile([128, WCOLS], F32R)
            nc.scalar.dma_start(out=wt, in_=w_h[:, :])
            for d in range(xgrp, nchunk, xgrp):
                dg = min(xgrp, nchunk - d)
                nc.sync.dma_start(
                    out=xall[:, d:d + dg, :], in_=xv[:, d:d + dg, :]
                )
            xTfull = singles.tile([128, rows], F32R)

            def wslice(off, i, width=128):
                return wt[:, off + i * width: off + (i + 1) * width]

            relu = mybir.ActivationFunctionType.Relu

            # dummy matmul: advances PE's observed DMA ticks so the
            # transpose-mode matmuls below each need <=1 sync wait
            # (walrus caps waits on the transpose/LW encoding).
            dtile = ps.tile([128, 2, 512], F32, tag="d")
            nc.tensor.matmul(
                dtile[:1, 0, :1], ident[:1, :1], xall[:1, :1, :1],
                start=True, stop=True,
            )

            def _body(_iv=None):
                # ---- transpose all of x upfront: [rows,128] -> xTfull [128, rows]
                for k0 in range(0, nchunk, 8):
                    grp = min(8, nchunk - k0)
                    tp = ps.tile([128, 2, 512], F32, tag="d")
                    for q in range(grp):
                        c = k0 + q
                        nc.tensor.transpose(
                            tp[:, q // 4, (q % 4) * 128:(q % 4 + 1) * 128],
                            xall[:, c, :], ident,
                        )
                    nc.scalar.copy(
                        xTfull[:, k0 * 128:(k0 + grp) * 128],
                        tp.rearrange("p b c -> p (b c)")[:, :grp * 128],
                    )

                for it in range(nt):
                    r0 = it * RT

                    # ---- conv1 + pool1: 16 structured matmuls
                    # B tiles hold the odd pool partners: ScalarE relus them
                    # to SBUF (only one DVE input may come from PSUM), then
                    # one DVE stt per pair fuses pool+bias+relu:
                    #   max(A+b, relu(B+b)) = relu(max(A,B)+b)
                    xT = xTfull[:, r0:r0 + RT]
                    act1 = a1_pool.tile([128, 8, RT], F32R)
                    for T0 in range(0, 8, 2):
                        for dT in range(2):
                            T = T0 + dT
                            p01 = ps.tile([128, 2, 512], F32, tag="d")
                            nc.tensor.matmul(
                                p01[:, 0, :RT], wslice(OFF_A1, 2 * T), xT,
                                start=True, stop=True,
                            )
                            nc.tensor.matmul(
                                p01[:, 1, :RT], wslice(OFF_A1, 2 * T + 1), xT,
                                start=True, stop=True,
                            )
                            nc.vector.reduce_max(
                                act1[:, T, :],
                                p01[:, :, :RT].transpose([0, 2, 1]),
                                axis=mybir.AxisListType.X, op=max_op,
                            )
                        if T0 in (2, 6):
                            h0 = T0 - 2
                            nc.vector.tensor_scalar(
                                act1[:, h0:h0 + 4, :], act1[:, h0:h0 + 4, :],
                                bt[:, 1:2], 0.0, op0=add_op, op1=max_op,
                            )

                    # ---- conv2: weight-major over u pairs (LDW reuse)
                    # tiles: pA = (bank A0, A1) -> act2 slots 2u0, 2u0+1;
                    #        pC = (bank B0, B1) -> act2 slots 2u0+2, 2u0+3.
                    # corrections are K=32 sub-matmuls on disjoint PE strips;
                    # adjacent (wm, wp) pairs run concurrently in the array
                    act2 = a2_pool.tile([128, 16, RT], F32R)
                    wm = wslice(OFF_W2, 1)
                    wp = wslice(OFF_W2, 3)
                    for u0 in range(0, 8, 2):
                        u1 = u0 + 1
                        pA = ps.tile([128, 2, 512], F32, tag="d")
                        pC = ps.tile([128, 2, 512], F32, tag="d")
                        nc.tensor.matmul(
                            pA[:, 0, :RT], wslice(OFF_W2, 0), act1[:, u0, :],
                            start=True, stop=(u0 == 0),
                        )
                        nc.tensor.matmul(
                            pC[:, 0, :RT], wslice(OFF_W2, 0), act1[:, u1, :],
                            start=True, stop=False,
                        )
                        nc.tensor.matmul(
                            pA[:, 1, :RT], wslice(OFF_W2, 2), act1[:, u0, :],
                            start=True, stop=False,
                        )
                        nc.tensor.matmul(
                            pC[:, 1, :RT], wslice(OFF_W2, 2), act1[:, u1, :],
                            start=True, stop=(u1 == 7),
                        )
                        if u0 > 0:
                            nc.tensor.matmul(
                                pA[:, 0, :RT], wm, act1[:, u0 - 1, :],
                                start=False, stop=True,
                            )
                        nc.tensor.matmul(
                            pA[:, 1, :RT], wp, act1[:, u1, :],
                            start=False, stop=True,
                        )
                        nc.scalar.activation(
                            act2[:, 2 * u0:2 * u0 + 2, :], pA[:, :, :RT], relu,
                            bias=bt[:, 3:4],
                        )
                        nc.tensor.matmul(
                            pC[:, 0, :RT], wm, act1[:, u0, :],
                            start=False, stop=True,
                        )
                        if u1 < 7:
                            nc.tensor.matmul(
                                pC[:, 1, :RT], wp, act1[:, u1 + 1, :],
                                start=False, stop=True,
                            )
                        nc.scalar.activation(
                            act2[:, 2 * u0 + 2:2 * u0 + 4, :], pC[:, :, :RT], relu,
                            bias=bt[:, 3:4],
                        )

                    # ---- conv3 + pool3: weight-major over b pairs (LDW
                    # reuse). Tiles: pO = (O_b0, O_b1) relu'd to SBUF by
                    # ScalarE (frees early), pE = (E_b0, E_b1) combined by
                    # one DVE stt into pooled act3. Corrections are K=64
                    # row-tiled; (wo2, we2) adjacent pairs run concurrently.
                    act3 = a3_pool.tile([128, 16, RT], F32R)
                    we2 = wslice(OFF_W3, 1)
                    wo2 = wslice(OFF_W3, 3)
                    for b0 in range(0, 16, 2):
                        b1 = b0 + 1
                        pX0 = ps.tile([128, 2, 512], F32, tag="d")
                        pX1 = ps.tile([128, 2, 512], F32, tag="d")
                        nc.tensor.matmul(
                            pX0[:, 0, :RT], wslice(OFF_W3, 0), act2[:, b0, :],
                            start=True, stop=(b0 == 0),
                        )
                        nc.tensor.matmul(
                            pX1[:, 0, :RT], wslice(OFF_W3, 0), act2[:, b1, :],
                            start=True, stop=False,
                        )
                        nc.tensor.matmul(
                            pX0[:, 1, :RT], wslice(OFF_W3, 2), act2[:, b0, :],
                            start=True, stop=False,
                        )
                        nc.tensor.matmul(
                            pX1[:, 1, :RT], wslice(OFF_W3, 2), act2[:, b1, :],
                            start=True, stop=(b1 == 15),
                        )
                        if b0 > 0:
                            nc.tensor.matmul(
                                pX0[:, 0, :RT], we2, act2[:, b0 - 1, :],
                                start=False, stop=True,
                            )
                        nc.tensor.matmul(
                            pX0[:, 1, :RT], wo2, act2[:, b1, :],
                            start=False, stop=True,
                        )
                        nc.vector.reduce_max(
                            act3[:, b0, :],
                            pX0[:, :, :RT].transpose([0, 2, 1]),
                            axis=mybir.AxisListType.X, op=max_op,
                        )
                        nc.tensor.matmul(
                            pX1[:, 0, :RT], we2, act2[:, b0, :],
                            start=False, stop=True,
                        )
                        if b1 < 15:
                            nc.tensor.matmul(
                                pX1[:, 1, :RT], wo2, act2[:, b1 + 1, :],
                                start=False, stop=True,
                            )
                        nc.vector.reduce_max(
                            act3[:, b1, :],
                            pX1[:, :, :RT].transpose([0, 2, 1]),
                            axis=mybir.AxisListType.X, op=max_op,
                        )
                        if b0 in (6, 14):
                            h0 = b0 - 6
                            nc.scalar.activation(
                                act3[:, h0:h0 + 8, :], act3[:, h0:h0 + 8, :], relu,
                                bias=bt[:, 5:6],
                            )

                    # ---- conv4: contract H; both halves per weight (LDW reuse)
                    a3v = act3[:, :, :].rearrange("p b (i h) -> p b i h", h=H)
                    q4 = ps.tile([128, 2, 512], F32, tag="d")
                    p4A = q4[:, 0, :]
                    p4B = q4[:, 1, :]
                    for h in range(H):
                        nc.tensor.matmul(
                            p4A, wslice(OFF_W4, h), a3v[:, 0:8, :, h],
                            start=(h == 0), stop=(h == H - 1),
                        )
                        nc.tensor.matmul(
                            p4B, wslice(OFF_W4, h), a3v[:, 8:16, :, h],
                            start=(h == 0), stop=(h == H - 1),
                        )
                    nc.scalar.activation(
                        act4[:, :, it * G:(it + 1) * G],
                        q4[:, :, :].rearrange("p s (b i) -> p (s b) i", b=8),
                        relu,
                        bias=bt[:, 7:8],
                    )

                # ---- FC: 16 accumulating matmuls over w positions
                qf = ps.tile([128, 2, 512], F32, tag="d")
                pfc = qf[:, 0, :]
                for w in range(16):
                    nc.tensor.matmul(
                        pfc[:10, :n_img],
                        wt[:, OFF_FC + w * 10: OFF_FC + (w + 1) * 10],
                        act4[:, w, :],
                        start=(w == 0), stop=(w == 15),
                    )
                fcout = singles.tile([16, B_CORE], F32)
                nc.vector.tensor_scalar_add(fcout[:10, :n_img], pfc[:10, :n_img], bt[:10, 8:9])
                nc.sync.dma_start(out=out_h[:, :], in_=fcout[:10, :n_img])

            if reps == 1:
                _body()
            else:
                with tc.For_i(0, reps, 1) as _i:
                    _body(_i)

    nc.finalize()
    return nc


def prep_consts(w1, b1, g1, be1, m1, v1, w2, b2, g2, be2, m2, v2,
                w3, b3, g3, be3, m3, v3, w4, b4, g4, be4, m4, v4, wfc, bfc):
    f = np.float32
    EPS = 1e-5

    def binz(w):
        return np.where(w >= 0, 1.0, -1.0).astype(f)

    def fold(g, be, m, v, bconv):
        a = (g / np.sqrt(v + EPS)).astype(f)
        beta = ((bconv - m) * a + be).astype(f)
        sig = np.where(a >= 0, 1.0, -1.0).astype(f)
        return sig, np.abs(a).astype(f), beta

    sig1, sc1, bi1 = fold(g1, be1, m1, v1, b1)
    sig2, sc2, bi2 = fold(g2, be2, m2, v2, b2)
    sig3, sc3, bi3 = fold(g3, be3, m3, v3, b3)
    sig4, sc4, bi4 = fold(g4, be4, m4, v4, b4)

    # carry-fold: y_l = a_l * relu(z_l + beta_l / a_l); the a_l scale is
    # multiplied into the next layer's (binarized) weights per input channel.
    bi1 = bi1 / sc1
    bi2 = bi2 / sc2
    bi3 = bi3 / sc3
    bi4 = bi4 / sc4

    W1e = sig1[:, None] * binz(w1)[:, 0, 0, :]            # [32, 9]
    W2e = sig2[:, None, None] * binz(w2)[:, :, 0, :] * sc1[None, :, None]
    W3e = sig3[:, None, None] * binz(w3)[:, :, 0, :] * sc2[None, :, None]
    W4e = sig4[:, None, None] * binz(w4)[:, :, :, 0] * sc3[None, :, None]
    FCe = binz(wfc).reshape(10, 128, 16) * sc4[None, :, None]

    wpack = np.zeros((128, WCOLS), f)
    # conv1 structured matrices: tile t16 = 2*T + b0, column m = p*32 + c
    for T in range(8):
        for b0 in range(2):
            t16 = 2 * T + b0
            blk = np.zeros((128, 128), f)
            for p in range(4):
                j = 8 * T + 2 * p + b0
                for t in range(9):
                    i = 2 * j + t - 4
                    if 0 <= i < 128:
                        blk[i, p * 32:(p + 1) * 32] = W1e[:, t]
            wpack[:, OFF_A1 + t16 * 128: OFF_A1 + (t16 + 1) * 128] = blk

    # conv2: Wa, Wm, Wo, Wp  (rows k = v*32 + ic, cols m = s*64 + oc)
    Wa = np.zeros((128, 128), f)
    for v in (0, 1):
        Wa[v * 32:(v + 1) * 32, 0:64] = W2e[:, :, v + 1].T
    for v in (0, 1, 2):
        Wa[v * 32:(v + 1) * 32, 64:128] = W2e[:, :, v].T
    Wm = np.zeros((128, 128), f)
    Wm[96:128, 0:64] = W2e[:, :, 0].T
    Wo = np.zeros((128, 128), f)
    for v in (1, 2, 3):
        Wo[v * 32:(v + 1) * 32, 0:64] = W2e[:, :, v - 1].T
    for v in (2, 3):
        Wo[v * 32:(v + 1) * 32, 64:128] = W2e[:, :, v - 2].T
    Wp = np.zeros((128, 128), f)
    Wp[0:32, 64:128] = W2e[:, :, 2].T
    for i, Wx in enumerate((Wa, Wm, Wo, Wp)):
        wpack[:, OFF_W2 + i * 128: OFF_W2 + (i + 1) * 128] = Wx

    # conv3: We1, We2, Wo1, Wo2  (rows k = s*64 + ic, cols oc)
    We1 = np.zeros((128, 128), f)
    We1[0:64, :] = W3e[:, :, 1].T
    We1[64:128, :] = W3e[:, :, 2].T
    We2 = np.zeros((128, 128), f)
    We2[64:128, :] = W3e[:, :, 0].T
    Wo1 = np.zeros((128, 128), f)
    Wo1[0:64, :] = W3e[:, :, 0].T
    Wo1[64:128, :] = W3e[:, :, 1].T
    Wo2 = np.zeros((128, 128), f)
    Wo2[0:64, :] = W3e[:, :, 2].T
    for i, Wx in enumerate((We1, We2, Wo1, Wo2)):
        wpack[:, OFF_W3 + i * 128: OFF_W3 + (i + 1) * 128] = Wx

    for h in range(H):
        wpack[:, OFF_W4 + h * 128: OFF_W4 + (h + 1) * 128] = W4e[:, :, h].T

    for w in range(16):
        wpack[:, OFF_FC + w * 10: OFF_FC + (w + 1) * 10] = FCe[:, :, w].T

    cpack = np.zeros((128, CCOLS), f)
    cpack[:, OFF_IDENT:OFF_IDENT + 128] = np.eye(128, dtype=f)
    cpack[:, OFF_BIAS + 0] = 1.0
    cpack[:, OFF_BIAS + 1] = np.tile(bi1, 4)
    cpack[:, OFF_BIAS + 2] = 1.0
    cpack[:, OFF_BIAS + 3] = np.tile(bi2, 2)
    cpack[:, OFF_BIAS + 4] = 1.0
    cpack[:, OFF_BIAS + 5] = bi3
    cpack[:, OFF_BIAS + 6] = 1.0
    cpack[:, OFF_BIAS + 7] = bi4
    cpack[:10, OFF_BIAS + 8] = bfc
    return wpack, cpack


_NC_CACHE = {}


def _get_nc(n_img=B_CORE):
    if n_img not in _NC_CACHE:
        _NC_CACHE[n_img] = build_nc(n_img)
    return _NC_CACHE[n_img]


_LDWOPT_PATCHED = False


def _patch_ldwopt():
    global _LDWOPT_PATCHED
    if _LDWOPT_PATCHED or not os.environ.get("KERNEL_LDWOPT"):
        return
    import concourse.bass_utils as _bu
    orig = _bu.run_command

    def run_command_ldwopt(cmd, cwd=None, **kw):
        cmd = [
            c.replace("--enable-ldw-opt=false", "--enable-ldw-opt=true")
            if isinstance(c, str) else c
            for c in cmd
        ]
        return orig(cmd, cwd=cwd, **kw)

    _bu.run_command = run_command_ldwopt
    _LDWOPT_PATCHED = True


def kernel(x, **params):
    from concourse.bass_utils import run_bass_kernel_spmd
    _patch_ldwopt()
    params = {k: np.asarray(v, dtype=np.float32) for k, v in params.items()}

    wpack, ident = prep_consts(**params)  # (bf16 weights, f32 cpack)
    x = np.ascontiguousarray(np.asarray(x, dtype=np.float32))
    nc = _get_nc()
    core_ids = list(range(N_CORES))
    in_maps = [
        {
            "x": x[i * B_CORE:(i + 1) * B_CORE].reshape(B_CORE * H, W),
            "wpack": wpack,
            "cpack": ident,
        }
        for i in core_ids
    ]
    trace = bool(int(os.environ.get("KERNEL_TRACE", "0")))
    res = run_bass_kernel_spmd(nc, in_maps, core_ids, trace=trace)
    if trace and res.exec_time_ns is not None:
        print(f"HW exec time: {res.exec_time_ns} ns")
    out = np.concatenate(
        [np.ascontiguousarray(res.results[i]["out"].T) for i in range(N_CORES)],
        axis=0,
    )
    return out.astype(np.float32)

